# revision 1
# baseline (speedup 1.0000x reference)
"""Trainium2 Bass kernel for nn_MultiHeadAttention_53463752900838.

Math (per batch element b, one NeuronCore each — pure data parallel over B=8):
  qkv = w_qkv @ x + b_qkv                     (3072, T)
  q,k,v per head h: (64, T);  q scaled by 1/8 (folded into weights on host)
  scores[t,h,g] = sum_d q[h,d,t] k[g,d,t]     per-timestep 16x16 Gram matrix
  attn = softmax over t  (per (h,g) pair)
  context[h,d,t] = sum_g attn[t,h,g] v[g,d,t]
  out = w_out @ context + b_out               (1024, T)

Kernel layout strategy (all bf16 matmuls, fp32 PSUM accumulation):
  Pass 1 (per 256-t span): project QKV in natural (o, t) orientation,
    marshal per-head blocks into
      QT (64d, (h,t)) / KT (64d, (g,t)) / VT (16g, (d,t))
    via SBUF->SBUF DMA; per-t 16x16 scores matmuls (lhsT=KT slice, rhs=QT
    slice); fused exp during PSUM evac on ScalarE; running Z sums.
    exp(S) and VT spill to DRAM.
  Pass 2 (per span): reload, normalize by 1/Z, per-t context matmuls with
    tile_position column tiling, re-marshal context to channel-major via
    SBUF->SBUF DMA, final projection as out^T (t, o), host transposes back.
"""

import os
import sys
import contextlib

import numpy as np
import ml_dtypes

for p in ("/opt/trn_rl_repo",):
    if p not in sys.path and os.path.isdir(p):
        sys.path.insert(0, p)

import concourse.bass as bass
import concourse.tile as tile
from concourse import mybir
from concourse.bass_utils import run_bass_kernel_spmd

F32 = mybir.dt.float32
BF16 = mybir.dt.bfloat16

N_CORES = 8
C = 1024
H = 16
DK = 64
OC3 = 3072


_WAITS2_OK = {
    "InstMatmult",
    "InstLdweights",
    "InstTensorCopy",
    "InstActivation",
    "InstTensorTensor",
    "InstTensorReduce",
    "InstDMACopy",
    "InstTensorScalarPtr",
    "InstMemset",
}


def _split_sync_waits(nc, limit=1):
    """walrus codegen rejects too many semaphore waits per instruction (CTRL
    class takes 1); hoist overflow waits onto NoOps inserted before the
    offending instruction. Compute/DMA instructions take 2."""
    counter = [0]
    n_split = 0
    for fn in nc.m.functions:
        for bb in fn.blocks:
            out = []
            for ins in bb.instructions:
                si = getattr(ins, "sync_info", None)
                waits = list(si.on_wait) if (si is not None and si.on_wait) else []
                if len(waits) > limit:
                    n_split += 1
                    extra, keep = waits[:-limit], waits[-limit:]
                    for i in range(0, len(extra), limit):
                        counter[0] += 1
                        out.append(
                            mybir.InstNoOp(
                                name=f"I-wsplit-{counter[0]}",
                                opcode="NoOp",
                                engine=ins.engine,
                                ins=[],
                                outs=[],
                                sync_info=mybir.SyncInfo(
                                    on_wait=list(extra[i : i + limit]), on_update=[]
                                ),
                            )
                        )
                    si.on_wait = keep
                out.append(ins)
            bb.instructions[:] = out
    return n_split


def build_kernel(T=4096, SPAN=256):
    NSPAN = T // SPAN
    nc = bass.Bass("TRN2", target_bir_lowering=False, debug=False)

    x_in = nc.dram_tensor("x", [C, T], BF16, kind="ExternalInput").ap()
    wq_in = nc.dram_tensor("wqT", [C, OC3], BF16, kind="ExternalInput").ap()
    bq_in = nc.dram_tensor("bqT", [1, OC3], BF16, kind="ExternalInput").ap()
    wo_in = nc.dram_tensor("woT", [C, C], BF16, kind="ExternalInput").ap()
    bo_in = nc.dram_tensor("boT", [1, C], BF16, kind="ExternalInput").ap()
    out_t = nc.dram_tensor("outT", [T, C], F32, kind="ExternalOutput").ap()
    # DRAM scratch: exp(scores) (g, (h,t)) and VT (g, (d,t)) per span
    se_d = nc.dram_tensor("se_d", [16, H * T], BF16).ap()
    vt_d = nc.dram_tensor("vt_d", [16, DK * T], BF16).ap()

    Exp = mybir.ActivationFunctionType.Exp
    Copy = mybir.ActivationFunctionType.Copy
    ADD = mybir.AluOpType.add
    MUL = mybir.AluOpType.mult

    with tile.TileContext(nc) as tc, contextlib.ExitStack() as octx:
        const = octx.enter_context(tc.tile_pool(name="const", bufs=1))
        wo_sb = []
        for k in range(8):
            w = const.tile([128, C], BF16, tag=f"wo{k}")
            nc.sync.dma_start(w[:], wo_in[k * 128 : (k + 1) * 128, :])
            wo_sb.append(w)
        bo_sb = const.tile([1, C], BF16, tag="bo")
        nc.sync.dma_start(bo_sb[:], bo_in)
        ones_t = const.tile([1, SPAN], BF16, tag="ones_t")
        nc.gpsimd.memset(ones_t[:], 1.0)
        ones128 = const.tile([1, 128], BF16, tag="ones128")
        nc.gpsimd.memset(ones128[:], 1.0)
        zacc = const.tile([16, 16], F32, tag="zacc")
        rrec = const.tile([16, 16], F32, tag="rrec")

        # ---------------- PASS 1 ----------------
        with contextlib.ExitStack() as ctx:
            wpool = ctx.enter_context(tc.tile_pool(name="wq", bufs=1))
            wq_sb = []
            for k in range(8):
                w = wpool.tile([128, OC3], BF16, tag=f"wq{k}")
                nc.sync.dma_start(w[:], wq_in[k * 128 : (k + 1) * 128, :])
                wq_sb.append(w)
            bq_sb = wpool.tile([1, OC3], BF16, tag="bq")
            nc.sync.dma_start(bq_sb[:], bq_in)

            xpool = ctx.enter_context(tc.tile_pool(name="x", bufs=2))
            stpool = ctx.enter_context(tc.tile_pool(name="stage", bufs=2))
            qkpool = ctx.enter_context(tc.tile_pool(name="qkt", bufs=2))
            vtpool = ctx.enter_context(tc.tile_pool(name="vt", bufs=1))
            sepool = ctx.enter_context(tc.tile_pool(name="se", bufs=2))
            zpool = ctx.enter_context(tc.tile_pool(name="zp", bufs=2))
            ps_a = ctx.enter_context(tc.tile_pool(name="psA", bufs=3, space="PSUM"))
            ps_s = ctx.enter_context(tc.tile_pool(name="psS", bufs=3, space="PSUM"))

            for s in range(NSPAN):
                t0 = s * SPAN
                xs = []
                for k in range(8):
                    xk = xpool.tile([128, SPAN], BF16, tag=f"x{k}")
                    nc.sync.dma_start(xk[:], x_in[k * 128 : (k + 1) * 128, t0 : t0 + SPAN])
                    xs.append(xk)

                qt = qkpool.tile([64, H * SPAN], BF16, tag="qt")
                kt = qkpool.tile([64, H * SPAN], BF16, tag="kt")
                vt = vtpool.tile([16, DK * SPAN], BF16, tag="vt")

                stages = {}
                for kind in range(3):  # 0=q, 1=k, 2=v
                    stages[kind] = stpool.tile(
                        [128, 8 * SPAN], BF16, tag=f"st{kind}", name=f"st{kind}"
                    )
                for m in range(24):
                    kind, mm = divmod(m, 8)
                    ps = ps_a.tile([128, SPAN], F32, tag="psA")
                    nc.tensor.matmul(
                        ps[:],
                        lhsT=bq_sb[0:1, m * 128 : (m + 1) * 128],
                        rhs=ones_t[:],
                        start=True,
                        stop=False,
                    )
                    for k in range(8):
                        nc.tensor.matmul(
                            ps[:],
                            lhsT=wq_sb[k][:, m * 128 : (m + 1) * 128],
                            rhs=xs[k][:],
                            start=False,
                            stop=(k == 7),
                        )
                    stg = stages[kind][:, mm * SPAN : (mm + 1) * SPAN]
                    if m % 2 == 0:
                        nc.vector.tensor_copy(stg, ps[:])
                    else:
                        nc.scalar.activation(stg, ps[:], Copy)
                # marshal: Q/K via 2 strided HWDGE copies each; V via 8 SWDGE
                for dst, kind in ((qt, 0), (kt, 1)):
                    src = stages[kind]
                    for par in range(2):
                        nc.sync.dma_start(
                            dst[0:64, :].rearrange(
                                "p (m par t) -> p m par t", m=8, par=2
                            )[:, :, par, :],
                            src[par * 64 : (par + 1) * 64, :]
                            .rearrange("p (m t) -> p m t", m=8),
                        )
                for mm in range(8):
                    nc.gpsimd.dma_start(
                        vt[2 * mm : 2 * mm + 2, :].rearrange("p (d t) -> p d t", d=DK),
                        stages[2][:, mm * SPAN : (mm + 1) * SPAN],
                    )

                qtv = qt[:].rearrange("p (h t) -> p t h", h=H)
                ktv = kt[:].rearrange("p (g t) -> p t g", g=H)
                se = sepool.tile([16, H * SPAN], BF16, tag="se")
                sev = se[:].rearrange("p (h t) -> p t h", h=H)
                for blk in range(SPAN // 32):
                    pss = ps_s.tile([16, 512], F32, tag="psS")
                    for s32 in range(32):
                        tl = blk * 32 + s32
                        nc.tensor.matmul(
                            pss[:, s32 * 16 : (s32 + 1) * 16],
                            lhsT=ktv[:, tl, :],
                            rhs=qtv[:, tl, :],
                            start=True,
                            stop=True,
                        )
                    nc.scalar.activation(
                        sev[:, blk * 32 : (blk + 1) * 32, :],
                        pss[:].rearrange("p (t h) -> p t h", h=H),
                        Exp,
                    )
                zp = zpool.tile([16, 16], F32, tag="zp")
                nc.vector.tensor_reduce(
                    zp[:],
                    se[:].rearrange("p (h t) -> p h t", h=H),
                    axis=mybir.AxisListType.X,
                    op=ADD,
                )
                if s == 0:
                    nc.vector.tensor_copy(zacc[:], zp[:])
                else:
                    nc.vector.tensor_tensor(out=zacc[:], in0=zacc[:], in1=zp[:], op=ADD)
                nc.sync.dma_start(se_d[:, s * H * SPAN : (s + 1) * H * SPAN], se[:])
                nc.sync.dma_start(vt_d[:, s * DK * SPAN : (s + 1) * DK * SPAN], vt[:])

            nc.vector.reciprocal(rrec[:], zacc[:])

        # ---------------- PASS 2 ----------------
        with contextlib.ExitStack() as ctx:
            sepool = ctx.enter_context(tc.tile_pool(name="se2", bufs=2))
            vtpool = ctx.enter_context(tc.tile_pool(name="vt2", bufs=2))
            apool = ctx.enter_context(tc.tile_pool(name="attn", bufs=2))
            cpool = ctx.enter_context(tc.tile_pool(name="csb", bufs=2))
            cnpool = ctx.enter_context(tc.tile_pool(name="cnat", bufs=1))
            opool = ctx.enter_context(tc.tile_pool(name="osb", bufs=2))
            ps_c = ctx.enter_context(tc.tile_pool(name="psC", bufs=4, space="PSUM"))
            ps_o = ctx.enter_context(tc.tile_pool(name="psO", bufs=3, space="PSUM"))

            rbc = rrec[:].unsqueeze(2).broadcast_to([16, 16, SPAN])

            NW = min(4, NSPAN)
            SPC = NW * SPAN  # context accumulation block (1024 t)
            for sb_ in range(NSPAN // NW):
                tB0 = sb_ * SPC
                # C_sb[32j+h, d*256 + w*64 + u]: t_in_block = w*SPAN + j*64 + u
                csb = cpool.tile([128, DK * 64 * NW], BF16, tag="csb")
                for w in range(NW):
                    s = sb_ * NW + w
                    se = sepool.tile([16, H * SPAN], BF16, tag="se2")
                    nc.sync.dma_start(se[:], se_d[:, s * H * SPAN : (s + 1) * H * SPAN])
                    vt = vtpool.tile([16, DK * SPAN], BF16, tag="vt2")
                    nc.sync.dma_start(vt[:], vt_d[:, s * DK * SPAN : (s + 1) * DK * SPAN])

                    at = apool.tile([16, H * SPAN], BF16, tag="attn")
                    nc.vector.tensor_tensor(
                        out=at[:].rearrange("p (h t) -> p h t", h=H),
                        in0=se[:].rearrange("p (h t) -> p h t", h=H),
                        in1=rbc,
                        op=MUL,
                    )

                    atv = at[:].rearrange("p (h t) -> p t h", h=H)
                    vtv = vt[:].rearrange("p (d t) -> p t d", d=DK)
                    for q in range(8):
                        psc = ps_c.tile([128, 512], F32, tag="psC")
                        for j in range(4):
                            for s8 in range(8):
                                tl = j * 64 + q * 8 + s8
                                nc.tensor.matmul(
                                    psc[32 * j : 32 * j + 16, s8 * 64 : (s8 + 1) * 64],
                                    lhsT=atv[:, tl, :],
                                    rhs=vtv[:, tl, :],
                                    start=True,
                                    stop=True,
                                    tile_position=(0, 32 * j),
                                )
                        csb_dst = csb[:].rearrange("p (d tj) -> p tj d", d=DK)[
                            :, w * 64 + q * 8 : w * 64 + (q + 1) * 8, :
                        ]
                        psc_src = psc[:].rearrange("p (s d) -> p s d", s=8)
                        if q % 2 == 0:
                            nc.vector.tensor_copy(csb_dst, psc_src)
                        else:
                            nc.scalar.activation(csb_dst, psc_src, Copy)

                # marshal: Cnat rows (h%2)*64+d, free = k*SPC + w*SPAN + j*64 + u
                cnat = cnpool.tile([128, 8 * SPC], BF16, tag="cnat")
                for j in range(4):
                    for k in range(8):
                        nc.gpsimd.dma_start(
                            cnat[:, :]
                            .rearrange("p (kk w j u) -> p kk w j u", kk=8, w=NW, j=4)[
                                :, k, :, j, :
                            ],
                            csb[32 * j + 2 * k : 32 * j + 2 * k + 2, :].rearrange(
                                "p (d w u) -> p d w u", d=DK, w=NW
                            ),
                        )

                for mt in range(SPC // 128):
                    for n in range(2):
                        pso = ps_o.tile([128, 512], F32, tag="psO")
                        nc.tensor.matmul(
                            pso[:],
                            lhsT=ones128[:],
                            rhs=bo_sb[0:1, n * 512 : (n + 1) * 512],
                            start=True,
                            stop=False,
                        )
                        for k in range(8):
                            nc.tensor.matmul(
                                pso[:],
                                lhsT=cnat[:, k * SPC + mt * 128 : k * SPC + mt * 128 + 128],
                                rhs=wo_sb[k][:, n * 512 : (n + 1) * 512],
                                start=False,
                                stop=(k == 7),
                            )
                        osb = opool.tile([128, 512], F32, tag="osb")
                        nc.scalar.activation(osb[:], pso[:], Copy)
                        nc.sync.dma_start(
                            out_t[tB0 + mt * 128 : tB0 + mt * 128 + 128, n * 512 : (n + 1) * 512],
                            osb[:],
                        )

    _split_sync_waits(nc, limit=1)
    return nc


_NC_CACHE = {}


def _get_nc(T, SPAN):
    key = (T, SPAN)
    if key not in _NC_CACHE:
        _NC_CACHE[key] = build_kernel(T, SPAN)
    return _NC_CACHE[key]


def _prep_weights(w_qkv, b_qkv, w_out, b_out):
    bf = ml_dtypes.bfloat16
    w3 = w_qkv.reshape(H, 192, C).astype(np.float32)
    qw = (w3[:, :DK, :] / 8.0).reshape(H * DK, C)
    kw = w3[:, DK : 2 * DK, :].reshape(H * DK, C)
    vw = w3[:, 2 * DK :, :].reshape(H * DK, C)
    wqT = np.concatenate([qw, kw, vw], axis=0).T.copy().astype(bf)  # (C, 3072)
    b3 = b_qkv.reshape(H, 192).astype(np.float32)
    bq = np.concatenate(
        [(b3[:, :DK] / 8.0).reshape(-1), b3[:, DK : 2 * DK].reshape(-1), b3[:, 2 * DK :].reshape(-1)]
    ).reshape(1, OC3).astype(bf)
    woT = w_out.T.copy().astype(bf)  # (C, C) rows = (h,d) h-major
    boT = b_out.reshape(1, C).astype(bf)
    return wqT, bq, woT, boT


def kernel(x, w_qkv, b_qkv, w_out, b_out, _trace=False, _span=256):
    B, _, T = x.shape
    assert B == N_CORES
    nc = _get_nc(T, _span)
    wqT, bq, woT, boT = _prep_weights(w_qkv, b_qkv, w_out, b_out)
    bf = ml_dtypes.bfloat16
    in_maps = []
    for b in range(B):
        in_maps.append(
            {
                "x": x[b].astype(bf),
                "wqT": wqT,
                "bqT": bq,
                "woT": woT,
                "boT": boT,
            }
        )
    res = run_bass_kernel_spmd(nc, in_maps, list(range(N_CORES)), trace=_trace)
    out = np.stack([res.results[b]["outT"].T for b in range(B)], axis=0)
    if _trace:
        kernel.last_exec_time_ns = res.exec_time_ns
        kernel.last_results = res
    return out.astype(np.float32)



# revision 2
# speedup vs baseline: 1.0492x; 1.0492x over previous
"""Trainium2 Bass kernel for nn_MultiHeadAttention_53463752900838 (v2).

Data-parallel over B=8 (one core per batch element). Per core:
  qkv = w_qkv @ x + b_qkv; per-t 16x16 cross-head scores; softmax over t;
  context; out = w_out @ context + b_out.

v2 layout strategy (t split into 8 stripes u = t//512, G = t%512):
  Pass 1 (per 256-t span, u = s//2 fixed per span):
    - single gather DMA for x span; 24x8 accumulating QKV matmuls (no bias
      matmuls -- bias added during PSUM evac via per-partition bias APs,
      rotated across Act/DVE/Pool engines);
    - Q/K marshal to (64d, (h,t)) via 4 strided HWDGE DMAs; per-t 16x16
      score matmuls; exp evac straight into persistent se_st[16u+g, 512h+G];
    - V evac into vsp, spilled to DRAM vnat_d; per-stripe vstk gather
      vstk[16u+g, 512d+G] issued as soon as its stripe's spills land;
    - per-span Z partial reduce on DVE into zacc[16u+g, h] (off critical
      path -- nothing on PE waits for it until pass 2).
  Pass 2:
    - fold zacc over stripes via tiny DMA, reciprocal, log-replicate to
      rrec_rep (128,16);
    - per 32-G chunk: normalize se into block-diagonal attn weights
      ad[16u+g, G'*128+16u+h] (zeros memset once per buffer); one 128x128
      ldweights + 64-col matmul per G computes ALL 8 stripes' 16x64
      contexts at once into psum[16u+h, d];
    - evac to ctx_sb[16u+h, 512d+G]; after all chunks: per-stripe marshal
      (8 DMAs, 1KB descriptors) to channel-major cn, then out-projection
      out^T[t, o] with bias matmul, evac to bf16, one store DMA per stripe.
"""

import os
import sys
import contextlib

import numpy as np
import ml_dtypes

for p in ("/opt/trn_rl_repo",):
    if p not in sys.path and os.path.isdir(p):
        sys.path.insert(0, p)

import concourse.bass as bass
import concourse.tile as tile
from concourse import mybir
from concourse.bass_utils import run_bass_kernel_spmd

F32 = mybir.dt.float32
BF16 = mybir.dt.bfloat16

N_CORES = 8
C = 1024
H = 16
DK = 64
OC3 = 3072


_WAITS2_OK = {
    "InstMatmult",
    "InstLdweights",
    "InstTensorCopy",
    "InstActivation",
    "InstTensorTensor",
    "InstTensorReduce",
    "InstDMACopy",
    "InstTensorScalarPtr",
    "InstMemset",
}


def _split_sync_waits(nc, limit=1):
    """walrus codegen rejects too many semaphore waits per instruction; hoist
    overflow waits onto NoOps inserted before the offending instruction."""
    counter = [0]
    n_split = 0
    for fn in nc.m.functions:
        for bb in fn.blocks:
            out = []
            for ins in bb.instructions:
                si = getattr(ins, "sync_info", None)
                waits = list(si.on_wait) if (si is not None and si.on_wait) else []
                if len(waits) > limit:
                    n_split += 1
                    extra, keep = waits[:-limit], waits[-limit:]
                    for i in range(0, len(extra), limit):
                        counter[0] += 1
                        out.append(
                            mybir.InstNoOp(
                                name=f"I-wsplit-{counter[0]}",
                                opcode="NoOp",
                                engine=ins.engine,
                                ins=[],
                                outs=[],
                                sync_info=mybir.SyncInfo(
                                    on_wait=list(extra[i : i + limit]), on_update=[]
                                ),
                            )
                        )
                    si.on_wait = keep
                out.append(ins)
            bb.instructions[:] = out
    return n_split


def build_kernel(T=4096, SPAN=256):
    NSPAN = T // SPAN
    S = T // 8            # stripe length (u = t // S)
    GC = 64               # G's per pass-2 chunk
    NCHUNK = S // GC
    nc = bass.Bass("TRN2", target_bir_lowering=False, debug=False)

    x_in = nc.dram_tensor("x", [C, T], BF16, kind="ExternalInput").ap()
    wq_in = nc.dram_tensor("wqT", [C, OC3], BF16, kind="ExternalInput").ap()
    bq_in = nc.dram_tensor("bqT", [128, 24], F32, kind="ExternalInput").ap()
    wo_in = nc.dram_tensor("woT", [C, C], BF16, kind="ExternalInput").ap()
    bo_in = nc.dram_tensor("boT", [1, C], BF16, kind="ExternalInput").ap()
    out_t = nc.dram_tensor("outT", [T, C], BF16, kind="ExternalOutput").ap()
    vnat_d = nc.dram_tensor("vnat_d", [C, T], BF16).ap()
    ctxd = nc.dram_tensor("ctxd", [C, T], BF16).ap()

    Exp = mybir.ActivationFunctionType.Exp
    Copy = mybir.ActivationFunctionType.Copy
    Identity = mybir.ActivationFunctionType.Identity
    ADD = mybir.AluOpType.add
    MUL = mybir.AluOpType.mult

    with tile.TileContext(nc) as tc, contextlib.ExitStack() as octx:
        const = octx.enter_context(tc.tile_pool(name="const", bufs=1))
        bq_sb = const.tile([128, 24], F32, tag="bq")
        nc.sync.dma_start(bq_sb[:], bq_in)
        bo_sb = const.tile([1, C], BF16, tag="bo")
        nc.sync.dma_start(bo_sb[:], bo_in)
        ones128 = const.tile([1, 128], BF16, tag="ones128")
        nc.gpsimd.memset(ones128[:], 1.0)
        zacc = const.tile([128, 16], F32, tag="zacc")
        zfold = const.tile([16, 128], F32, tag="zfold")
        z16 = const.tile([16, 16], F32, tag="z16")
        r16 = const.tile([16, 16], F32, tag="r16")
        rrec = const.tile([128, 16], BF16, tag="rrec")
        se_st = const.tile([128, H * S], BF16, tag="se_st")    # [16u+g, 512h+G]
        vstk = const.tile([128, DK * S], BF16, tag="vstk")     # [16u+g, 512d+G]

        # ---------------- PASS 1 ----------------
        with contextlib.ExitStack() as ctx:
            wpool = ctx.enter_context(tc.tile_pool(name="wq", bufs=1))
            wq_sb = []
            for k in range(8):
                w = wpool.tile([128, OC3], BF16, tag=f"wq{k}")
                nc.sync.dma_start(w[:], wq_in[k * 128 : (k + 1) * 128, :])
                wq_sb.append(w)

            xpool = ctx.enter_context(tc.tile_pool(name="x", bufs=2))
            stpool = ctx.enter_context(tc.tile_pool(name="stage", bufs=2))
            vppool = ctx.enter_context(tc.tile_pool(name="vsp", bufs=2))
            sepool = ctx.enter_context(tc.tile_pool(name="sesp", bufs=1))
            qkpool = ctx.enter_context(tc.tile_pool(name="qkt", bufs=2))
            ps_a = ctx.enter_context(tc.tile_pool(name="psA", bufs=3, space="PSUM"))
            ps_s = ctx.enter_context(tc.tile_pool(name="psS", bufs=3, space="PSUM"))

            xs_t = {}
            bk_t = {}

            def emit_x(s):
                t0 = s * SPAN
                xk = xpool.tile([128, 8 * SPAN], BF16, tag="xs")
                nc.sync.dma_start(
                    xk[:].rearrange("p (k t) -> p k t", k=8),
                    x_in.rearrange("(k p) t -> p k t", k=8)[:, :, t0 : t0 + SPAN],
                )
                xs_t[s] = xk

            def emit_front(s):
                t0 = s * SPAN
                xk = xs_t.pop(s)
                stq = stpool.tile([128, 8 * SPAN], BF16, tag="stq")
                stk = stpool.tile([128, 8 * SPAN], BF16, tag="stk")
                vsp = vppool.tile([128, 8 * SPAN], BF16, tag="vsp")
                dsts = {0: stq, 1: stk, 2: vsp}
                for m in range(24):
                    kind, mm = divmod(m, 8)
                    ps = ps_a.tile([128, SPAN], F32, tag="psA")
                    for k in range(8):
                        nc.tensor.matmul(
                            ps[:],
                            lhsT=wq_sb[k][:, m * 128 : (m + 1) * 128],
                            rhs=xk[:, k * SPAN : (k + 1) * SPAN],
                            start=(k == 0),
                            stop=(k == 7),
                        )
                    dst = dsts[kind][:, mm * SPAN : (mm + 1) * SPAN]
                    bias = bq_sb[:, m : m + 1]
                    if m % 2 == 0:
                        nc.scalar.activation(dst, ps[:], Identity, bias=bias)
                    else:
                        nc.vector.tensor_scalar(dst, ps[:], bias, None, op0=ADD)
                # marshal Q/K: qt/kt[d, h*SPAN+t], h = 2m+a
                qt = qkpool.tile([64, H * SPAN], BF16, tag="qt")
                kt = qkpool.tile([64, H * SPAN], BF16, tag="kt")
                for dst, src in ((qt, stq), (kt, stk)):
                    for a in range(2):
                        nc.sync.dma_start(
                            dst[0:64, :].rearrange(
                                "p (m a t) -> p m a t", m=8, a=2
                            )[:, :, a, :],
                            src[a * 64 : (a + 1) * 64, :].rearrange(
                                "p (m t) -> p m t", m=8
                            ),
                        )
                bk_t[s] = (qt, kt)
                # V spill to DRAM (natural channel-major (g,d))
                nc.sync.dma_start(
                    vnat_d.rearrange("(m p) t -> p m t", m=8)[:, :, t0 : t0 + SPAN],
                    vsp[:].rearrange("p (m t) -> p m t", m=8),
                )

            def emit_vstk(u):
                nc.sync.dma_start(
                    vstk[16 * u : 16 * u + 16, :].rearrange("p (d G) -> p d G", d=DK),
                    vnat_d.rearrange("(g d) t -> g d t", g=16)[:, :, S * u : S * (u + 1)],
                )

            def emit_back(s):
                u = s // 2
                Goff = (s % 2) * SPAN
                qt, kt = bk_t.pop(s)
                qtv = qt[:].rearrange("p (h t) -> p t h", h=H)
                ktv = kt[:].rearrange("p (h t) -> p t h", h=H)
                se_sp = sepool.tile([16, H * SPAN], BF16, tag="sesp")
                sev = se_sp[:].rearrange("p (h t) -> p t h", h=H)
                for blk in range(SPAN // 32):
                    pss = ps_s.tile([16, 512], F32, tag="psS")
                    for i in range(32):
                        tl = blk * 32 + i
                        nc.tensor.matmul(
                            pss[:, i * 16 : (i + 1) * 16],
                            lhsT=ktv[:, tl, :],
                            rhs=qtv[:, tl, :],
                            start=True,
                            stop=True,
                        )
                    g0 = blk * 32
                    nc.scalar.activation(
                        sev[:, g0 : g0 + 32, :],
                        pss[:].rearrange("p (t h) -> p t h", h=H),
                        Exp,
                    )
                # lane-shift staging -> se_st[16u+g, 512h + Goff+t]
                nc.sync.dma_start(
                    se_st[16 * u : 16 * u + 16, :].rearrange("p (h G) -> p h G", h=H)[
                        :, :, Goff : Goff + SPAN
                    ],
                    se_sp[:].rearrange("p (h t) -> p h t", h=H),
                )
                # Z reduce per stripe-pair (32-aligned partitions), after the
                # pair's 4 spans complete; off the PE critical path.
                if s % 4 == 3:
                    w = s // 4
                    nc.vector.tensor_reduce(
                        zacc[32 * w : 32 * w + 32, :],
                        se_st[32 * w : 32 * w + 32, :].rearrange("p (h G) -> p h G", h=H),
                        axis=mybir.AxisListType.X,
                        op=ADD,
                    )

            emit_x(0)
            for s in range(NSPAN):
                if s + 1 < NSPAN:
                    emit_x(s + 1)
                emit_front(s)
                if s % 2 == 1:
                    emit_vstk(s // 2)
                if s >= 1:
                    emit_back(s - 1)
            emit_back(NSPAN - 1)

            # Z fold across stripes + reciprocal + replicate
            for u in range(8):
                nc.sync.dma_start(
                    zfold[:].rearrange("p (h u) -> p h u", u=8)[:, :, u],
                    zacc[16 * u : 16 * u + 16, :],
                )
            nc.vector.tensor_reduce(
                z16[:], zfold[:].rearrange("p (h u) -> p h u", u=8),
                axis=mybir.AxisListType.X, op=ADD,
            )
            nc.vector.reciprocal(r16[:], z16[:])
            nc.vector.tensor_copy(rrec[0:16, :], r16[:])
            nc.sync.dma_start(rrec[16:32, :], rrec[0:16, :])
            nc.sync.dma_start(rrec[32:64, :], rrec[0:32, :])
            nc.sync.dma_start(rrec[64:128, :], rrec[0:64, :])

        # ---------------- PASS 2 ----------------
        with contextlib.ExitStack() as ctx:
            wopool = ctx.enter_context(tc.tile_pool(name="wo", bufs=1))
            wo_sb = []
            for k in range(8):
                w = wopool.tile([128, C], BF16, tag=f"wo{k}")
                nc.sync.dma_start(w[:], wo_in[k * 128 : (k + 1) * 128, :])
                wo_sb.append(w)

            cxpool = ctx.enter_context(tc.tile_pool(name="cx", bufs=1))
            ctx_sb = cxpool.tile([128, DK * S], BF16, tag="ctx_sb")  # [16u+h, 512d+G]
            ps_c = ctx.enter_context(tc.tile_pool(name="psC", bufs=3, space="PSUM"))
            ps_o = ctx.enter_context(tc.tile_pool(name="psO", bufs=3, space="PSUM"))
            chunk_ctx = ctx.enter_context(contextlib.ExitStack())
            adpool = chunk_ctx.enter_context(tc.tile_pool(name="ad", bufs=1))
            ad_bufs = [
                adpool.tile([128, GC * 128], BF16, tag="ad0", name="ad0"),
                adpool.tile([128, GC * 128], BF16, tag="ad1", name="ad1"),
            ]
            nc.vector.memset(ad_bufs[0][:], 0.0)
            nc.gpsimd.memset(ad_bufs[1][:], 0.0)
            atpool = chunk_ctx.enter_context(tc.tile_pool(name="attn_ch", bufs=2))

            csv = ctx_sb[:].rearrange("p (d G) -> p G d", d=DK)
            vsv = vstk[:].rearrange("p (d G) -> p G d", d=DK)

            def emit_chunk(ch):
                ad = ad_bufs[ch % 2]
                adv = ad[:].rearrange("p (G w) -> p G w", w=128)
                # normalize on 32-aligned stripe pairs into compact attn_ch,
                # then scatter diag blocks into ad via SWDGE (any-offset) DMAs
                at = atpool.tile([128, GC * H], BF16, tag="attn_ch")
                atv = at[:].rearrange("p (G h) -> p G h", h=H)
                for w in range(4):
                    nc.vector.tensor_tensor(
                        out=atv[32 * w : 32 * w + 32, :, :],
                        in0=se_st[32 * w : 32 * w + 32, :].rearrange(
                            "p (h G) -> p G h", h=H
                        )[:, ch * GC : (ch + 1) * GC, :],
                        in1=rrec[32 * w : 32 * w + 32, :]
                        .unsqueeze(1)
                        .broadcast_to([32, GC, 16]),
                        op=MUL,
                    )
                for u in range(8):
                    q = nc.gpsimd if u % 2 == 0 else nc.sync
                    q.dma_start(
                        adv[16 * u : 16 * u + 16, :, 16 * u : 16 * u + 16],
                        atv[16 * u : 16 * u + 16, :, :],
                    )
                for bk in range(GC // 8):
                    psc = ps_c.tile([128, 512], F32, tag="psC")
                    for i in range(8):
                        Gp = bk * 8 + i
                        G = ch * GC + Gp
                        nc.tensor.matmul(
                            psc[:, i * 64 : (i + 1) * 64],
                            lhsT=ad[:, Gp * 128 : (Gp + 1) * 128],
                            rhs=vsv[:, G, :],
                            start=True,
                            stop=True,
                        )
                    nc.scalar.activation(
                        csv[:, ch * GC + bk * 8 : ch * GC + (bk + 1) * 8, :],
                        psc[:].rearrange("p (g8 d) -> p g8 d", d=DK),
                        Copy,
                    )

            def emit_spill(sc):
                # ctx_sb superchunk -> DRAM ctxd, channel-major (h*64+d, t)
                g0 = sc * (S // 2)
                for u in range(8):
                    nc.sync.dma_start(
                        ctxd.rearrange("(h d) t -> h d t", h=16)[
                            :, :, S * u + g0 : S * u + g0 + S // 2
                        ],
                        ctx_sb[16 * u : 16 * u + 16, :].rearrange(
                            "p (d G) -> p d G", d=DK
                        )[:, :, g0 : g0 + S // 2],
                    )

            def emit_wave(j):
                # gather stripe j from DRAM ctxd into channel-major cn
                cn = cnpool.tile([128, 8 * S], BF16, tag="cn")
                nc.sync.dma_start(
                    cn[:].rearrange("p (k t) -> p k t", k=8),
                    ctxd.rearrange("(k p) t -> p k t", k=8)[:, :, S * j : S * (j + 1)],
                )
                osb = ospool.tile([128, 8 * 512], BF16, tag="osb")
                for mt in range(4):
                    for n in range(2):
                        pso = ps_o.tile([128, 512], F32, tag="psO")
                        nc.tensor.matmul(
                            pso[:],
                            lhsT=ones128[:],
                            rhs=bo_sb[0:1, n * 512 : (n + 1) * 512],
                            start=True,
                            stop=False,
                        )
                        for k in range(8):
                            nc.tensor.matmul(
                                pso[:],
                                lhsT=cn[:, k * S + mt * 128 : k * S + mt * 128 + 128],
                                rhs=wo_sb[k][:, n * 512 : (n + 1) * 512],
                                start=False,
                                stop=(k == 7),
                            )
                        dst = osb[:, (mt * 2 + n) * 512 : (mt * 2 + n + 1) * 512]
                        if (mt * 2 + n) % 2 == 0:
                            nc.scalar.activation(dst, pso[:], Copy)
                        else:
                            nc.vector.tensor_copy(dst, pso[:])
                nc.sync.dma_start(
                    out_t[S * j : S * (j + 1), :].rearrange(
                        "(mt p) o -> p mt o", mt=4
                    ),
                    osb[:].rearrange("p (mt o) -> p mt o", mt=4),
                )

            for ch in range(NCHUNK):
                emit_chunk(ch)
                if ch == NCHUNK // 2 - 1:
                    emit_spill(0)
            emit_spill(1)
            chunk_ctx.close()
            cnpool = ctx.enter_context(tc.tile_pool(name="cn", bufs=2))
            ospool = ctx.enter_context(tc.tile_pool(name="osb", bufs=1))
            for j in range(8):
                emit_wave(j)

    _split_sync_waits(nc, limit=1)
    return nc


_NC_CACHE = {}


def _get_nc(T, SPAN):
    key = (T, SPAN)
    if key not in _NC_CACHE:
        _NC_CACHE[key] = build_kernel(T, SPAN)
    return _NC_CACHE[key]


def _prep_weights(w_qkv, b_qkv, w_out, b_out):
    bf = ml_dtypes.bfloat16
    w3 = w_qkv.reshape(H, 192, C).astype(np.float32)
    qw = (w3[:, :DK, :] / 8.0).reshape(H * DK, C)
    kw = w3[:, DK : 2 * DK, :].reshape(H * DK, C)
    vw = w3[:, 2 * DK :, :].reshape(H * DK, C)
    wqT = np.concatenate([qw, kw, vw], axis=0).T.copy().astype(bf)  # (C, 3072)
    b3 = b_qkv.reshape(H, 192).astype(np.float32)
    bq = np.concatenate(
        [(b3[:, :DK] / 8.0).reshape(-1), b3[:, DK : 2 * DK].reshape(-1), b3[:, 2 * DK :].reshape(-1)]
    )
    bq_d = bq.reshape(24, 128).T.copy().astype(np.float32)          # (128, 24)
    woT = w_out.T.copy().astype(bf)   # rows = context channels (h,d) h-major
    boT = b_out.reshape(1, C).astype(bf)
    return wqT, bq_d, woT, boT


def kernel(x, w_qkv, b_qkv, w_out, b_out, _trace=False, _span=256):
    B, _, T = x.shape
    assert B == N_CORES
    nc = _get_nc(T, _span)
    wqT, bq_d, woT, boT = _prep_weights(w_qkv, b_qkv, w_out, b_out)
    bf = ml_dtypes.bfloat16
    in_maps = []
    for b in range(B):
        in_maps.append(
            {
                "x": x[b].astype(bf),
                "wqT": wqT,
                "bqT": bq_d,
                "woT": woT,
                "boT": boT,
            }
        )
    res = run_bass_kernel_spmd(nc, in_maps, list(range(N_CORES)), trace=_trace)
    out = np.stack(
        [res.results[b]["outT"].astype(np.float32).T for b in range(B)], axis=0
    )
    if _trace:
        kernel.last_exec_time_ns = res.exec_time_ns
        kernel.last_results = res
    return out


# revision 3
# speedup vs baseline: 1.0589x; 1.0092x over previous
"""Trainium2 Bass kernel for nn_MultiHeadAttention_53463752900838 (v2).

Data-parallel over B=8 (one core per batch element). Per core:
  qkv = w_qkv @ x + b_qkv; per-t 16x16 cross-head scores; softmax over t;
  context; out = w_out @ context + b_out.

v2 layout strategy (t split into 8 stripes u = t//512, G = t%512):
  Pass 1 (per 256-t span, u = s//2 fixed per span):
    - single gather DMA for x span; 24x8 accumulating QKV matmuls (no bias
      matmuls -- bias added during PSUM evac via per-partition bias APs,
      rotated across Act/DVE/Pool engines);
    - Q/K marshal to (64d, (h,t)) via 4 strided HWDGE DMAs; per-t 16x16
      score matmuls; exp evac straight into persistent se_st[16u+g, 512h+G];
    - V evac into vsp, spilled to DRAM vnat_d; per-stripe vstk gather
      vstk[16u+g, 512d+G] issued as soon as its stripe's spills land;
    - per-span Z partial reduce on DVE into zacc[16u+g, h] (off critical
      path -- nothing on PE waits for it until pass 2).
  Pass 2:
    - fold zacc over stripes via tiny DMA, reciprocal, log-replicate to
      rrec_rep (128,16);
    - per 32-G chunk: normalize se into block-diagonal attn weights
      ad[16u+g, G'*128+16u+h] (zeros memset once per buffer); one 128x128
      ldweights + 64-col matmul per G computes ALL 8 stripes' 16x64
      contexts at once into psum[16u+h, d];
    - evac to ctx_sb[16u+h, 512d+G]; after all chunks: per-stripe marshal
      (8 DMAs, 1KB descriptors) to channel-major cn, then out-projection
      out^T[t, o] with bias matmul, evac to bf16, one store DMA per stripe.
"""

import os
import sys
import contextlib

import numpy as np
import ml_dtypes

for p in ("/opt/trn_rl_repo",):
    if p not in sys.path and os.path.isdir(p):
        sys.path.insert(0, p)

import concourse.bass as bass
import concourse.tile as tile
from concourse import mybir
from concourse.bass_utils import run_bass_kernel_spmd

F32 = mybir.dt.float32
BF16 = mybir.dt.bfloat16

N_CORES = 8
C = 1024
H = 16
DK = 64
OC3 = 3072


_WAITS2_OK = {
    "InstMatmult",
    "InstLdweights",
    "InstTensorCopy",
    "InstActivation",
    "InstTensorTensor",
    "InstTensorReduce",
    "InstDMACopy",
    "InstTensorScalarPtr",
    "InstMemset",
}


def _split_sync_waits(nc, limit=1):
    """walrus codegen rejects too many semaphore waits per instruction; hoist
    overflow waits onto NoOps inserted before the offending instruction."""
    counter = [0]
    n_split = 0
    for fn in nc.m.functions:
        for bb in fn.blocks:
            out = []
            for ins in bb.instructions:
                si = getattr(ins, "sync_info", None)
                waits = list(si.on_wait) if (si is not None and si.on_wait) else []
                if len(waits) > limit:
                    n_split += 1
                    extra, keep = waits[:-limit], waits[-limit:]
                    for i in range(0, len(extra), limit):
                        counter[0] += 1
                        out.append(
                            mybir.InstNoOp(
                                name=f"I-wsplit-{counter[0]}",
                                opcode="NoOp",
                                engine=ins.engine,
                                ins=[],
                                outs=[],
                                sync_info=mybir.SyncInfo(
                                    on_wait=list(extra[i : i + limit]), on_update=[]
                                ),
                            )
                        )
                    si.on_wait = keep
                out.append(ins)
            bb.instructions[:] = out
    return n_split


def build_kernel(T=4096, SPAN=256):
    NSPAN = T // SPAN
    S = T // 8            # stripe length (u = t // S)
    GC = 64               # G's per pass-2 chunk
    NCHUNK = S // GC
    nc = bass.Bass("TRN2", target_bir_lowering=False, debug=False)

    x_in = nc.dram_tensor("x", [C, T], BF16, kind="ExternalInput").ap()
    wq_in = nc.dram_tensor("wqT", [C, OC3], BF16, kind="ExternalInput").ap()
    bq_in = nc.dram_tensor("bqT", [128, 24], F32, kind="ExternalInput").ap()
    wo_in = nc.dram_tensor("woT", [C, C], BF16, kind="ExternalInput").ap()
    bo_in = nc.dram_tensor("boT", [1, C], BF16, kind="ExternalInput").ap()
    out_t = nc.dram_tensor("outT", [T, C], BF16, kind="ExternalOutput").ap()
    vnat_d = nc.dram_tensor("vnat_d", [C, T], BF16).ap()
    ctxd = nc.dram_tensor("ctxd", [C, T], BF16).ap()

    Exp = mybir.ActivationFunctionType.Exp
    Copy = mybir.ActivationFunctionType.Copy
    Identity = mybir.ActivationFunctionType.Identity
    ADD = mybir.AluOpType.add
    MUL = mybir.AluOpType.mult

    with tile.TileContext(nc) as tc, contextlib.ExitStack() as octx:
        const = octx.enter_context(tc.tile_pool(name="const", bufs=1))
        bq_sb = const.tile([128, 24], F32, tag="bq")
        nc.sync.dma_start(bq_sb[:], bq_in)
        bo_sb = const.tile([1, C], BF16, tag="bo")
        nc.sync.dma_start(bo_sb[:], bo_in)
        ones128 = const.tile([1, 128], BF16, tag="ones128")
        nc.gpsimd.memset(ones128[:], 1.0)
        zacc2 = const.tile([16, 128], F32, tag="zacc2")   # [g, u*16+h]
        ztmp = const.tile([16, 16], F32, tag="ztmp")
        z16 = const.tile([16, 16], F32, tag="z16")
        r16 = const.tile([16, 16], F32, tag="r16")
        rrec = const.tile([128, 16], BF16, tag="rrec")
        se_st = const.tile([128, H * S], BF16, tag="se_st")    # [16u+g, 512h+G]
        vstk = const.tile([128, DK * S], BF16, tag="vstk")     # [16u+g, 512d+G]
        wo_sb = []
        for k in range(8):
            w = const.tile([128, C], BF16, tag=f"wo{k}", name=f"wo{k}")
            wo_sb.append(w)

        # ---------------- PASS 1 ----------------
        with contextlib.ExitStack() as ctx:
            wpool = ctx.enter_context(tc.tile_pool(name="wq", bufs=1))
            wq_sb = []
            for k in range(8):
                w = wpool.tile([128, OC3], BF16, tag=f"wq{k}", name=f"wq{k}")
                wq_sb.append(w)
            # column-sliced loads: m-blocks 0..7 usable after the first 8 DMAs
            for sl in range(3):
                for k in range(8):
                    nc.sync.dma_start(
                        wq_sb[k][:, sl * 1024 : (sl + 1) * 1024],
                        wq_in[k * 128 : (k + 1) * 128, sl * 1024 : (sl + 1) * 1024],
                    )

            xpool = ctx.enter_context(tc.tile_pool(name="x", bufs=2))
            stpool = ctx.enter_context(tc.tile_pool(name="stage", bufs=2))
            vppool = ctx.enter_context(tc.tile_pool(name="vsp", bufs=2))
            sepool = ctx.enter_context(tc.tile_pool(name="sesp", bufs=1))
            qkpool = ctx.enter_context(tc.tile_pool(name="qkt", bufs=1))
            ps_a = ctx.enter_context(tc.tile_pool(name="psA", bufs=3, space="PSUM"))
            ps_s = ctx.enter_context(tc.tile_pool(name="psS", bufs=3, space="PSUM"))

            xs_t = {}
            bk_t = {}

            def emit_x(s):
                t0 = s * SPAN
                xk = xpool.tile([128, 8 * SPAN], BF16, tag="xs")
                nc.sync.dma_start(
                    xk[:].rearrange("p (k t) -> p k t", k=8),
                    x_in.rearrange("(k p) t -> p k t", k=8)[:, :, t0 : t0 + SPAN],
                )
                xs_t[s] = xk

            def emit_front(s):
                t0 = s * SPAN
                xk = xs_t.pop(s)
                stq = stpool.tile([128, 8 * SPAN], BF16, tag="stq")
                stk = stpool.tile([128, 8 * SPAN], BF16, tag="stk")
                vsp = vppool.tile([128, 8 * SPAN], BF16, tag="vsp")
                dsts = {0: stq, 1: stk, 2: vsp}
                for m in range(24):
                    kind, mm = divmod(m, 8)
                    ps = ps_a.tile([128, SPAN], F32, tag="psA")
                    for k in range(8):
                        nc.tensor.matmul(
                            ps[:],
                            lhsT=wq_sb[k][:, m * 128 : (m + 1) * 128],
                            rhs=xk[:, k * SPAN : (k + 1) * SPAN],
                            start=(k == 0),
                            stop=(k == 7),
                        )
                    dst = dsts[kind][:, mm * SPAN : (mm + 1) * SPAN]
                    bias = bq_sb[:, m : m + 1]
                    if m % 2 == 0:
                        nc.scalar.activation(dst, ps[:], Identity, bias=bias)
                    else:
                        nc.vector.tensor_scalar(dst, ps[:], bias, None, op0=ADD)
                # marshal Q/K: per time-half tiles qt/kt[d, h*HSPAN+t], h = 2m+a
                HSPAN = SPAN // 2
                halves = []
                for hf in range(2):
                    qt = qkpool.tile([64, H * HSPAN], BF16, tag=f"qt{hf}", name=f"qt{hf}")
                    kt = qkpool.tile([64, H * HSPAN], BF16, tag=f"kt{hf}", name=f"kt{hf}")
                    for dst, src in ((qt, stq), (kt, stk)):
                        for a in range(2):
                            nc.sync.dma_start(
                                dst[0:64, :].rearrange(
                                    "p (m a t) -> p m a t", m=8, a=2
                                )[:, :, a, :],
                                src[a * 64 : (a + 1) * 64, :].rearrange(
                                    "p (m t) -> p m t", m=8
                                )[:, :, hf * HSPAN : (hf + 1) * HSPAN],
                            )
                    halves.append((qt, kt))
                bk_t[s] = halves
                # V spill to DRAM (natural channel-major (g,d))
                nc.sync.dma_start(
                    vnat_d.rearrange("(m p) t -> p m t", m=8)[:, :, t0 : t0 + SPAN],
                    vsp[:].rearrange("p (m t) -> p m t", m=8),
                )

            def emit_vstk(u):
                nc.sync.dma_start(
                    vstk[16 * u : 16 * u + 16, :].rearrange("p (d G) -> p d G", d=DK),
                    vnat_d.rearrange("(g d) t -> g d t", g=16)[:, :, S * u : S * (u + 1)],
                )

            def emit_back(s):
                u = s // 2
                Goff = (s % 2) * SPAN
                halves = bk_t.pop(s)
                HSPAN = SPAN // 2
                qkviews = [
                    (qt[:].rearrange("p (h t) -> p t h", h=H),
                     kt[:].rearrange("p (h t) -> p t h", h=H))
                    for qt, kt in halves
                ]
                se_sp = sepool.tile([16, H * SPAN], BF16, tag="sesp")
                sev = se_sp[:].rearrange("p (h t) -> p t h", h=H)
                for blk in range(SPAN // 32):
                    pss = ps_s.tile([16, 512], F32, tag="psS")
                    for i in range(32):
                        tl = blk * 32 + i
                        qtv, ktv = qkviews[tl // HSPAN]
                        tlh = tl % HSPAN
                        nc.tensor.matmul(
                            pss[:, i * 16 : (i + 1) * 16],
                            lhsT=ktv[:, tlh, :],
                            rhs=qtv[:, tlh, :],
                            start=True,
                            stop=True,
                        )
                    g0 = blk * 32
                    nc.scalar.activation(
                        sev[:, g0 : g0 + 32, :],
                        pss[:].rearrange("p (t h) -> p t h", h=H),
                        Exp,
                    )
                # lane-shift staging -> se_st[16u+g, 512h + Goff+t]
                nc.sync.dma_start(
                    se_st[16 * u : 16 * u + 16, :].rearrange("p (h G) -> p h G", h=H)[
                        :, :, Goff : Goff + SPAN
                    ],
                    se_sp[:].rearrange("p (h t) -> p h t", h=H),
                )
                # Z partial from the staging tile (partitions 0:16, aligned);
                # last span split per 64-t block to shorten the tail chain.
                zdst = zacc2[:, 16 * u : 16 * u + 16]
                spv = se_sp[:].rearrange("p (h t) -> p h t", h=H)
                if s == NSPAN - 1:
                    for q4 in range(4):
                        nc.vector.tensor_reduce(
                            ztmp[:], spv[:, :, q4 * (SPAN // 4) : (q4 + 1) * (SPAN // 4)],
                            axis=mybir.AxisListType.X, op=ADD,
                        )
                        nc.vector.tensor_tensor(out=zdst, in0=zdst, in1=ztmp[:], op=ADD)
                else:
                    nc.vector.tensor_reduce(
                        ztmp[:], spv, axis=mybir.AxisListType.X, op=ADD)
                    if s % 2 == 0:
                        nc.vector.tensor_copy(zdst, ztmp[:])
                    else:
                        nc.vector.tensor_tensor(out=zdst, in0=zdst, in1=ztmp[:], op=ADD)
            emit_x(0)
            for s in range(NSPAN):
                if s + 1 < NSPAN:
                    emit_x(s + 1)
                emit_front(s)
                if s == 2:
                    for k in range(8):
                        nc.sync.dma_start(wo_sb[k][:], wo_in[k * 128 : (k + 1) * 128, :])
                if s % 2 == 1:
                    emit_vstk(s // 2)
                if s >= 1:
                    emit_back(s - 1)
            emit_back(NSPAN - 1)

            # total Z over stripes + reciprocal + replicate
            nc.vector.tensor_reduce(
                z16[:], zacc2[:].rearrange("p (u h) -> p h u", u=8),
                axis=mybir.AxisListType.X, op=ADD,
            )
            nc.vector.reciprocal(r16[:], z16[:])
            nc.vector.tensor_copy(rrec[0:16, :], r16[:])
            for uu in range(1, 8):
                q = nc.sync if uu % 2 else nc.gpsimd
                q.dma_start(rrec[16 * uu : 16 * uu + 16, :], rrec[0:16, :])

        # ---------------- PASS 2 ----------------
        with contextlib.ExitStack() as ctx:


            cxpool = ctx.enter_context(tc.tile_pool(name="cx", bufs=1))
            ctx_sb = cxpool.tile([128, DK * S], BF16, tag="ctx_sb")  # [16u+h, 512d+G]
            ps_c = ctx.enter_context(tc.tile_pool(name="psC", bufs=3, space="PSUM"))
            ps_o = ctx.enter_context(tc.tile_pool(name="psO", bufs=3, space="PSUM"))
            chunk_ctx = ctx.enter_context(contextlib.ExitStack())
            adpool = chunk_ctx.enter_context(tc.tile_pool(name="ad", bufs=1))
            ad_bufs = [
                adpool.tile([128, GC * 128], BF16, tag="ad0", name="ad0"),
                adpool.tile([128, GC * 128], BF16, tag="ad1", name="ad1"),
            ]
            nc.gpsimd.memset(ad_bufs[0][:, : GC * 64], 0.0)
            nc.gpsimd.memset(ad_bufs[0][:, GC * 64 :], 0.0)
            atpool = chunk_ctx.enter_context(tc.tile_pool(name="attn_ch", bufs=2))

            csv = ctx_sb[:].rearrange("p (d G) -> p G d", d=DK)
            vsv = vstk[:].rearrange("p (d G) -> p G d", d=DK)

            def emit_chunk(ch):
                ad = ad_bufs[ch % 2]
                adv = ad[:].rearrange("p (G w) -> p G w", w=128)
                # normalize on 32-aligned stripe pairs into compact attn_ch,
                # then scatter diag blocks into ad via SWDGE (any-offset) DMAs
                at = atpool.tile([128, GC * H], BF16, tag="attn_ch")
                atv = at[:].rearrange("p (G h) -> p G h", h=H)
                for w in range(4):
                    nc.vector.tensor_tensor(
                        out=atv[32 * w : 32 * w + 32, :, :],
                        in0=se_st[32 * w : 32 * w + 32, :].rearrange(
                            "p (h G) -> p G h", h=H
                        )[:, ch * GC : (ch + 1) * GC, :],
                        in1=rrec[32 * w : 32 * w + 32, :]
                        .unsqueeze(1)
                        .broadcast_to([32, GC, 16]),
                        op=MUL,
                    )
                for u in range(8):
                    if ch == 0:
                        q = nc.scalar if u % 2 == 0 else nc.sync
                    else:
                        q = nc.gpsimd if u % 2 == 0 else nc.sync
                    q.dma_start(
                        adv[16 * u : 16 * u + 16, :, 16 * u : 16 * u + 16],
                        atv[16 * u : 16 * u + 16, :, :],
                    )
                for bk in range(GC // 8):
                    psc = ps_c.tile([128, 512], F32, tag="psC")
                    for i in range(8):
                        Gp = bk * 8 + i
                        G = ch * GC + Gp
                        nc.tensor.matmul(
                            psc[:, i * 64 : (i + 1) * 64],
                            lhsT=ad[:, Gp * 128 : (Gp + 1) * 128],
                            rhs=vsv[:, G, :],
                            start=True,
                            stop=True,
                        )
                    nc.scalar.activation(
                        csv[:, ch * GC + bk * 8 : ch * GC + (bk + 1) * 8, :],
                        psc[:].rearrange("p (g8 d) -> p g8 d", d=DK),
                        Copy,
                    )

            def emit_spill(ch0, ch1, ulist=range(8)):
                # ctx_sb G-range [ch0*GC, (ch1+1)*GC) -> DRAM ctxd (h*64+d, t)
                g0, g1 = ch0 * GC, (ch1 + 1) * GC
                for u in ulist:
                    nc.sync.dma_start(
                        ctxd.rearrange("(h d) t -> h d t", h=16)[
                            :, :, S * u + g0 : S * u + g1
                        ],
                        ctx_sb[16 * u : 16 * u + 16, :].rearrange(
                            "p (d G) -> p d G", d=DK
                        )[:, :, g0:g1],
                    )

            cn_t = {}

            def emit_gather(j):
                cn = cnpool.tile([128, 8 * S], BF16, tag="cn")
                nc.sync.dma_start(
                    cn[:].rearrange("p (k t) -> p k t", k=8),
                    ctxd.rearrange("(k p) t -> p k t", k=8)[:, :, S * j : S * (j + 1)],
                )
                cn_t[j] = cn

            def emit_wave(j):
                cn = cn_t.pop(j)
                osb = ospool.tile([128, 8 * 512], BF16, tag="osb")
                for mt in range(4):
                    for n in range(2):
                        pso = ps_o.tile([128, 512], F32, tag="psO")
                        nc.tensor.matmul(
                            pso[:],
                            lhsT=ones128[:],
                            rhs=bo_sb[0:1, n * 512 : (n + 1) * 512],
                            start=True,
                            stop=False,
                        )
                        for k in range(8):
                            nc.tensor.matmul(
                                pso[:],
                                lhsT=cn[:, k * S + mt * 128 : k * S + mt * 128 + 128],
                                rhs=wo_sb[k][:, n * 512 : (n + 1) * 512],
                                start=False,
                                stop=(k == 7),
                            )
                        dst = osb[:, (mt * 2 + n) * 512 : (mt * 2 + n + 1) * 512]
                        if (mt * 2 + n) % 2 == 0:
                            nc.scalar.activation(dst, pso[:], Copy)
                        else:
                            nc.vector.tensor_copy(dst, pso[:])
                nc.sync.dma_start(
                    out_t[S * j : S * (j + 1), :].rearrange(
                        "(mt p) o -> p mt o", mt=4
                    ),
                    osb[:].rearrange("p (mt o) -> p mt o", mt=4),
                )

            for ch in range(NCHUNK):
                emit_chunk(ch)
                if ch == 0:
                    nc.gpsimd.memset(ad_bufs[1][:, : GC * 64], 0.0)
                    nc.gpsimd.memset(ad_bufs[1][:, GC * 64 :], 0.0)
                if NCHUNK // 2 - 1 <= ch < NCHUNK // 2 + 3:
                    uu = 2 * (ch - (NCHUNK // 2 - 1))
                    emit_spill(0, NCHUNK // 2 - 1, [uu, uu + 1])
            chunk_ctx.close()
            cnpool = ctx.enter_context(tc.tile_pool(name="cn", bufs=2))
            ospool = ctx.enter_context(tc.tile_pool(name="osb", bufs=2))
            emit_spill(NCHUNK // 2, NCHUNK - 1, [0])
            emit_gather(0)
            emit_spill(NCHUNK // 2, NCHUNK - 1, [1])
            emit_gather(1)
            emit_spill(NCHUNK // 2, NCHUNK - 1, range(2, 8))
            for j in range(8):
                if j + 2 < 8:
                    emit_gather(j + 2)
                emit_wave(j)

    _split_sync_waits(nc, limit=1)
    return nc


_NC_CACHE = {}


def _get_nc(T, SPAN):
    key = (T, SPAN)
    if key not in _NC_CACHE:
        _NC_CACHE[key] = build_kernel(T, SPAN)
    return _NC_CACHE[key]


def _prep_weights(w_qkv, b_qkv, w_out, b_out):
    bf = ml_dtypes.bfloat16
    w3 = w_qkv.reshape(H, 192, C).astype(np.float32)
    qw = (w3[:, :DK, :] / 8.0).reshape(H * DK, C)
    kw = w3[:, DK : 2 * DK, :].reshape(H * DK, C)
    vw = w3[:, 2 * DK :, :].reshape(H * DK, C)
    wqT = np.concatenate([qw, kw, vw], axis=0).T.copy().astype(bf)  # (C, 3072)
    b3 = b_qkv.reshape(H, 192).astype(np.float32)
    bq = np.concatenate(
        [(b3[:, :DK] / 8.0).reshape(-1), b3[:, DK : 2 * DK].reshape(-1), b3[:, 2 * DK :].reshape(-1)]
    )
    bq_d = bq.reshape(24, 128).T.copy().astype(np.float32)          # (128, 24)
    woT = w_out.T.copy().astype(bf)   # rows = context channels (h,d) h-major
    boT = b_out.reshape(1, C).astype(bf)
    return wqT, bq_d, woT, boT


def kernel(x, w_qkv, b_qkv, w_out, b_out, _trace=False, _span=256):
    B, _, T = x.shape
    assert B == N_CORES
    nc = _get_nc(T, _span)
    wqT, bq_d, woT, boT = _prep_weights(w_qkv, b_qkv, w_out, b_out)
    bf = ml_dtypes.bfloat16
    in_maps = []
    for b in range(B):
        in_maps.append(
            {
                "x": x[b].astype(bf),
                "wqT": wqT,
                "bqT": bq_d,
                "woT": woT,
                "boT": boT,
            }
        )
    res = run_bass_kernel_spmd(nc, in_maps, list(range(N_CORES)), trace=_trace)
    out = np.stack(
        [res.results[b]["outT"].astype(np.float32).T for b in range(B)], axis=0
    )
    if _trace:
        kernel.last_exec_time_ns = res.exec_time_ns
        kernel.last_results = res
    return out


# revision 4
# speedup vs baseline: 1.0705x; 1.0109x over previous
"""Trainium2 Bass kernel for nn_MultiHeadAttention_53463752900838 (v2).

Data-parallel over B=8 (one core per batch element). Per core:
  qkv = w_qkv @ x + b_qkv; per-t 16x16 cross-head scores; softmax over t;
  context; out = w_out @ context + b_out.

v2 layout strategy (t split into 8 stripes u = t//512, G = t%512):
  Pass 1 (per 256-t span, u = s//2 fixed per span):
    - single gather DMA for x span; 24x8 accumulating QKV matmuls (no bias
      matmuls -- bias added during PSUM evac via per-partition bias APs,
      rotated across Act/DVE/Pool engines);
    - Q/K marshal to (64d, (h,t)) via 4 strided HWDGE DMAs; per-t 16x16
      score matmuls; exp evac straight into persistent se_st[16u+g, 512h+G];
    - V evac into vsp, spilled to DRAM vnat_d; per-stripe vstk gather
      vstk[16u+g, 512d+G] issued as soon as its stripe's spills land;
    - per-span Z partial reduce on DVE into zacc[16u+g, h] (off critical
      path -- nothing on PE waits for it until pass 2).
  Pass 2:
    - fold zacc over stripes via tiny DMA, reciprocal, log-replicate to
      rrec_rep (128,16);
    - per 32-G chunk: normalize se into block-diagonal attn weights
      ad[16u+g, G'*128+16u+h] (zeros memset once per buffer); one 128x128
      ldweights + 64-col matmul per G computes ALL 8 stripes' 16x64
      contexts at once into psum[16u+h, d];
    - evac to ctx_sb[16u+h, 512d+G]; after all chunks: per-stripe marshal
      (8 DMAs, 1KB descriptors) to channel-major cn, then out-projection
      out^T[t, o] with bias matmul, evac to bf16, one store DMA per stripe.
"""

import os
import sys
import contextlib

import numpy as np
import ml_dtypes

for p in ("/opt/trn_rl_repo",):
    if p not in sys.path and os.path.isdir(p):
        sys.path.insert(0, p)

import concourse.bass as bass
import concourse.tile as tile
from concourse import mybir
from concourse.bass_utils import run_bass_kernel_spmd

F32 = mybir.dt.float32
BF16 = mybir.dt.bfloat16

N_CORES = 8
C = 1024
H = 16
DK = 64
OC3 = 3072


_WAITS2_OK = {
    "InstMatmult",
    "InstLdweights",
    "InstTensorCopy",
    "InstActivation",
    "InstTensorTensor",
    "InstTensorReduce",
    "InstDMACopy",
    "InstTensorScalarPtr",
    "InstMemset",
}


def _split_sync_waits(nc, limit=1):
    """walrus codegen rejects too many semaphore waits per instruction; hoist
    overflow waits onto NoOps inserted before the offending instruction."""
    counter = [0]
    n_split = 0
    for fn in nc.m.functions:
        for bb in fn.blocks:
            out = []
            for ins in bb.instructions:
                si = getattr(ins, "sync_info", None)
                waits = list(si.on_wait) if (si is not None and si.on_wait) else []
                if len(waits) > limit:
                    n_split += 1
                    extra, keep = waits[:-limit], waits[-limit:]
                    for i in range(0, len(extra), limit):
                        counter[0] += 1
                        out.append(
                            mybir.InstNoOp(
                                name=f"I-wsplit-{counter[0]}",
                                opcode="NoOp",
                                engine=ins.engine,
                                ins=[],
                                outs=[],
                                sync_info=mybir.SyncInfo(
                                    on_wait=list(extra[i : i + limit]), on_update=[]
                                ),
                            )
                        )
                    si.on_wait = keep
                out.append(ins)
            bb.instructions[:] = out
    return n_split


def build_kernel(T=4096, SPAN=256):
    NSPAN = T // SPAN
    S = T // 8            # stripe length (u = t // S)
    GC = 64               # G's per pass-2 chunk
    NCHUNK = S // GC
    nc = bass.Bass("TRN2", target_bir_lowering=False, debug=False)

    x_in = nc.dram_tensor("x", [C, T], BF16, kind="ExternalInput").ap()
    wq_in = nc.dram_tensor("wqT", [C, OC3], BF16, kind="ExternalInput").ap()
    bq_in = nc.dram_tensor("bqT", [128, 24], F32, kind="ExternalInput").ap()
    wo_in = nc.dram_tensor("woT", [C, C], BF16, kind="ExternalInput").ap()
    bo_in = nc.dram_tensor("boT", [1, C], BF16, kind="ExternalInput").ap()
    out_t = nc.dram_tensor("outT", [T, C], BF16, kind="ExternalOutput").ap()
    vnat_d = nc.dram_tensor("vnat_d", [C, T], BF16).ap()
    ctxd = nc.dram_tensor("ctxd", [C, T], BF16).ap()

    Exp = mybir.ActivationFunctionType.Exp
    Copy = mybir.ActivationFunctionType.Copy
    Identity = mybir.ActivationFunctionType.Identity
    ADD = mybir.AluOpType.add
    MUL = mybir.AluOpType.mult

    with tile.TileContext(nc) as tc, contextlib.ExitStack() as octx:
        const = octx.enter_context(tc.tile_pool(name="const", bufs=1))
        bq_sb = const.tile([128, 24], F32, tag="bq")
        nc.sync.dma_start(bq_sb[:], bq_in)
        bo_sb = const.tile([1, C], BF16, tag="bo")
        nc.sync.dma_start(bo_sb[:], bo_in)
        ones128 = const.tile([1, 128], BF16, tag="ones128")
        nc.gpsimd.memset(ones128[:], 1.0)
        zacc2 = const.tile([16, 128], F32, tag="zacc2")   # [g, u*16+h]
        ztmp = const.tile([16, 16], F32, tag="ztmp")
        z16 = const.tile([16, 16], F32, tag="z16")
        r16 = const.tile([16, 16], F32, tag="r16")
        rrec = const.tile([128, 16], BF16, tag="rrec")
        se_st = const.tile([128, H * S], BF16, tag="se_st")    # [16u+g, 512h+G]
        vstk = const.tile([128, DK * S], BF16, tag="vstk")     # [16u+g, 512d+G]
        wo_sb = []
        for k in range(8):
            w = const.tile([128, C], BF16, tag=f"wo{k}", name=f"wo{k}")
            wo_sb.append(w)

        # ---------------- PASS 1 ----------------
        with contextlib.ExitStack() as ctx:
            wpool = ctx.enter_context(tc.tile_pool(name="wq", bufs=1))
            wq_sb = []
            for k in range(8):
                w = wpool.tile([128, OC3], BF16, tag=f"wq{k}", name=f"wq{k}")
                wq_sb.append(w)
            def emit_wq_loads():
                # column-sliced: m-blocks 0..7 usable after the first 8 DMAs
                for sl in range(3):
                    for k in range(8):
                        nc.sync.dma_start(
                            wq_sb[k][:, sl * 1024 : (sl + 1) * 1024],
                            wq_in[k * 128 : (k + 1) * 128, sl * 1024 : (sl + 1) * 1024],
                        )

            xpool = ctx.enter_context(tc.tile_pool(name="x", bufs=2))
            stpool = ctx.enter_context(tc.tile_pool(name="stage", bufs=2))
            vppool = ctx.enter_context(tc.tile_pool(name="vsp", bufs=2))
            sepool = ctx.enter_context(tc.tile_pool(name="sesp", bufs=1))
            qkpool = ctx.enter_context(tc.tile_pool(name="qkt", bufs=1))
            ps_a = ctx.enter_context(tc.tile_pool(name="psA", bufs=3, space="PSUM"))
            ps_s = ctx.enter_context(tc.tile_pool(name="psS", bufs=3, space="PSUM"))

            xs_t = {}
            bk_t = {}

            def emit_x(s):
                t0 = s * SPAN
                xk = xpool.tile([128, 8 * SPAN], BF16, tag="xs")
                nc.sync.dma_start(
                    xk[:].rearrange("p (k t) -> p k t", k=8),
                    x_in.rearrange("(k p) t -> p k t", k=8)[:, :, t0 : t0 + SPAN],
                )
                xs_t[s] = xk

            def emit_front(s):
                t0 = s * SPAN
                xk = xs_t.pop(s)
                stq = stpool.tile([128, 8 * SPAN], BF16, tag="stq")
                stk = stpool.tile([128, 8 * SPAN], BF16, tag="stk")
                vsp = vppool.tile([128, 8 * SPAN], BF16, tag="vsp")
                dsts = {0: stq, 1: stk, 2: vsp}
                for m in range(24):
                    kind, mm = divmod(m, 8)
                    ps = ps_a.tile([128, SPAN], F32, tag="psA")
                    for k in range(8):
                        nc.tensor.matmul(
                            ps[:],
                            lhsT=wq_sb[k][:, m * 128 : (m + 1) * 128],
                            rhs=xk[:, k * SPAN : (k + 1) * SPAN],
                            start=(k == 0),
                            stop=(k == 7),
                        )
                    dst = dsts[kind][:, mm * SPAN : (mm + 1) * SPAN]
                    bias = bq_sb[:, m : m + 1]
                    if m % 2 == 0:
                        nc.scalar.activation(dst, ps[:], Identity, bias=bias)
                    else:
                        nc.vector.tensor_scalar(dst, ps[:], bias, None, op0=ADD)
                # marshal Q/K: per time-half tiles qt/kt[d, h*HSPAN+t], h = 2m+a
                HSPAN = SPAN // 2
                halves = []
                for hf in range(2):
                    qt = qkpool.tile([64, H * HSPAN], BF16, tag=f"qt{hf}", name=f"qt{hf}")
                    kt = qkpool.tile([64, H * HSPAN], BF16, tag=f"kt{hf}", name=f"kt{hf}")
                    for dst, src in ((qt, stq), (kt, stk)):
                        for a in range(2):
                            nc.sync.dma_start(
                                dst[0:64, :].rearrange(
                                    "p (m a t) -> p m a t", m=8, a=2
                                )[:, :, a, :],
                                src[a * 64 : (a + 1) * 64, :].rearrange(
                                    "p (m t) -> p m t", m=8
                                )[:, :, hf * HSPAN : (hf + 1) * HSPAN],
                            )
                    halves.append((qt, kt))
                bk_t[s] = halves
                # V spill to DRAM (natural channel-major (g,d))
                nc.sync.dma_start(
                    vnat_d.rearrange("(m p) t -> p m t", m=8)[:, :, t0 : t0 + SPAN],
                    vsp[:].rearrange("p (m t) -> p m t", m=8),
                )

            def emit_vstk(u):
                nc.sync.dma_start(
                    vstk[16 * u : 16 * u + 16, :].rearrange("p (d G) -> p d G", d=DK),
                    vnat_d.rearrange("(g d) t -> g d t", g=16)[:, :, S * u : S * (u + 1)],
                )

            def emit_back(s):
                u = s // 2
                Goff = (s % 2) * SPAN
                halves = bk_t.pop(s)
                HSPAN = SPAN // 2
                qkviews = [
                    (qt[:].rearrange("p (h t) -> p t h", h=H),
                     kt[:].rearrange("p (h t) -> p t h", h=H))
                    for qt, kt in halves
                ]
                se_sp = sepool.tile([16, H * SPAN], BF16, tag="sesp")
                sev = se_sp[:].rearrange("p (h t) -> p t h", h=H)
                for blk in range(SPAN // 32):
                    pss = ps_s.tile([16, 512], F32, tag="psS")
                    for i in range(32):
                        tl = blk * 32 + i
                        qtv, ktv = qkviews[tl // HSPAN]
                        tlh = tl % HSPAN
                        nc.tensor.matmul(
                            pss[:, i * 16 : (i + 1) * 16],
                            lhsT=ktv[:, tlh, :],
                            rhs=qtv[:, tlh, :],
                            start=True,
                            stop=True,
                        )
                    g0 = blk * 32
                    nc.scalar.activation(
                        sev[:, g0 : g0 + 32, :],
                        pss[:].rearrange("p (t h) -> p t h", h=H),
                        Exp,
                    )
                # lane-shift staging -> se_st[16u+g, 512h + Goff+t]
                nc.sync.dma_start(
                    se_st[16 * u : 16 * u + 16, :].rearrange("p (h G) -> p h G", h=H)[
                        :, :, Goff : Goff + SPAN
                    ],
                    se_sp[:].rearrange("p (h t) -> p h t", h=H),
                )
                # Z partial from the staging tile (partitions 0:16, aligned);
                # last span split per 64-t block to shorten the tail chain.
                zdst = zacc2[:, 16 * u : 16 * u + 16]
                spv = se_sp[:].rearrange("p (h t) -> p h t", h=H)
                if s == NSPAN - 1:
                    for q4 in range(4):
                        nc.vector.tensor_reduce(
                            ztmp[:], spv[:, :, q4 * (SPAN // 4) : (q4 + 1) * (SPAN // 4)],
                            axis=mybir.AxisListType.X, op=ADD,
                        )
                        nc.vector.tensor_tensor(out=zdst, in0=zdst, in1=ztmp[:], op=ADD)
                else:
                    nc.vector.tensor_reduce(
                        ztmp[:], spv, axis=mybir.AxisListType.X, op=ADD)
                    if s % 2 == 0:
                        nc.vector.tensor_copy(zdst, ztmp[:])
                    else:
                        nc.vector.tensor_tensor(out=zdst, in0=zdst, in1=ztmp[:], op=ADD)
            emit_x(0)
            emit_x(1)
            emit_wq_loads()
            for s in range(NSPAN):
                if s + 2 < NSPAN:
                    emit_x(s + 2)
                emit_front(s)
                if s == 2:
                    for k in range(8):
                        nc.sync.dma_start(wo_sb[k][:], wo_in[k * 128 : (k + 1) * 128, :])
                if s % 2 == 1:
                    emit_vstk(s // 2)
                if s >= 1:
                    emit_back(s - 1)
            emit_back(NSPAN - 1)

            # total Z over stripes + reciprocal + replicate
            nc.vector.tensor_reduce(
                z16[:], zacc2[:].rearrange("p (u h) -> p h u", u=8),
                axis=mybir.AxisListType.X, op=ADD,
            )
            nc.vector.reciprocal(r16[:], z16[:])
            nc.vector.tensor_copy(rrec[0:16, :], r16[:])
            for uu in range(1, 8):
                q = nc.sync if uu % 2 else nc.gpsimd
                q.dma_start(rrec[16 * uu : 16 * uu + 16, :], rrec[0:16, :])

        # ---------------- PASS 2 ----------------
        with contextlib.ExitStack() as ctx:


            cxpool = ctx.enter_context(tc.tile_pool(name="cx", bufs=1))
            ctx_sb = cxpool.tile([128, DK * S], BF16, tag="ctx_sb")  # [16u+h, 512d+G]
            ps_c = ctx.enter_context(tc.tile_pool(name="psC", bufs=3, space="PSUM"))
            ps_o = ctx.enter_context(tc.tile_pool(name="psO", bufs=3, space="PSUM"))
            chunk_ctx = ctx.enter_context(contextlib.ExitStack())
            adpool = chunk_ctx.enter_context(tc.tile_pool(name="ad", bufs=1))
            ad_bufs = [
                adpool.tile([128, GC * 128], BF16, tag="ad0", name="ad0"),
                adpool.tile([128, GC * 128], BF16, tag="ad1", name="ad1"),
            ]
            nc.gpsimd.memset(ad_bufs[0][:, : GC * 64], 0.0)
            nc.gpsimd.memset(ad_bufs[0][:, GC * 64 :], 0.0)
            atpool = chunk_ctx.enter_context(tc.tile_pool(name="attn_ch", bufs=2))

            csv = ctx_sb[:].rearrange("p (d G) -> p G d", d=DK)
            vsv = vstk[:].rearrange("p (d G) -> p G d", d=DK)

            def emit_chunk(ch):
                ad = ad_bufs[ch % 2]
                adv = ad[:].rearrange("p (G w) -> p G w", w=128)
                # normalize on 32-aligned stripe pairs into compact attn_ch,
                # then scatter diag blocks into ad via SWDGE (any-offset) DMAs
                at = atpool.tile([128, GC * H], BF16, tag="attn_ch")
                atv = at[:].rearrange("p (G h) -> p G h", h=H)
                for w in range(4):
                    nc.vector.tensor_tensor(
                        out=atv[32 * w : 32 * w + 32, :, :],
                        in0=se_st[32 * w : 32 * w + 32, :].rearrange(
                            "p (h G) -> p G h", h=H
                        )[:, ch * GC : (ch + 1) * GC, :],
                        in1=rrec[32 * w : 32 * w + 32, :]
                        .unsqueeze(1)
                        .broadcast_to([32, GC, 16]),
                        op=MUL,
                    )
                for u in range(8):
                    if ch == 0:
                        q = nc.scalar if u % 2 == 0 else nc.sync
                    else:
                        q = nc.gpsimd if u % 2 == 0 else nc.sync
                    q.dma_start(
                        adv[16 * u : 16 * u + 16, :, 16 * u : 16 * u + 16],
                        atv[16 * u : 16 * u + 16, :, :],
                    )
                for bk in range(GC // 8):
                    psc = ps_c.tile([128, 512], F32, tag="psC")
                    for i in range(8):
                        Gp = bk * 8 + i
                        G = ch * GC + Gp
                        nc.tensor.matmul(
                            psc[:, i * 64 : (i + 1) * 64],
                            lhsT=ad[:, Gp * 128 : (Gp + 1) * 128],
                            rhs=vsv[:, G, :],
                            start=True,
                            stop=True,
                        )
                    nc.scalar.activation(
                        csv[:, ch * GC + bk * 8 : ch * GC + (bk + 1) * 8, :],
                        psc[:].rearrange("p (g8 d) -> p g8 d", d=DK),
                        Copy,
                    )

            def emit_spill(ch0, ch1, ulist=range(8)):
                # ctx_sb G-range [ch0*GC, (ch1+1)*GC) -> DRAM ctxd (h*64+d, t)
                g0, g1 = ch0 * GC, (ch1 + 1) * GC
                for u in ulist:
                    nc.sync.dma_start(
                        ctxd.rearrange("(h d) t -> h d t", h=16)[
                            :, :, S * u + g0 : S * u + g1
                        ],
                        ctx_sb[16 * u : 16 * u + 16, :].rearrange(
                            "p (d G) -> p d G", d=DK
                        )[:, :, g0:g1],
                    )

            cn_t = {}

            def emit_gather(j):
                cn = cnpool.tile([128, 8 * S], BF16, tag="cn")
                nc.sync.dma_start(
                    cn[:].rearrange("p (k t) -> p k t", k=8),
                    ctxd.rearrange("(k p) t -> p k t", k=8)[:, :, S * j : S * (j + 1)],
                )
                cn_t[j] = cn

            def emit_wave(j):
                cn = cn_t.pop(j)
                osb = ospool.tile([128, 8 * 512], BF16, tag="osb")
                for mt in range(4):
                    for n in range(2):
                        pso = ps_o.tile([128, 512], F32, tag="psO")
                        nc.tensor.matmul(
                            pso[:],
                            lhsT=ones128[:],
                            rhs=bo_sb[0:1, n * 512 : (n + 1) * 512],
                            start=True,
                            stop=False,
                        )
                        for k in range(8):
                            nc.tensor.matmul(
                                pso[:],
                                lhsT=cn[:, k * S + mt * 128 : k * S + mt * 128 + 128],
                                rhs=wo_sb[k][:, n * 512 : (n + 1) * 512],
                                start=False,
                                stop=(k == 7),
                            )
                        dst = osb[:, (mt * 2 + n) * 512 : (mt * 2 + n + 1) * 512]
                        if (mt * 2 + n) % 2 == 0:
                            nc.scalar.activation(dst, pso[:], Copy)
                        else:
                            nc.vector.tensor_copy(dst, pso[:])
                nc.sync.dma_start(
                    out_t[S * j : S * (j + 1), :].rearrange(
                        "(mt p) o -> p mt o", mt=4
                    ),
                    osb[:].rearrange("p (mt o) -> p mt o", mt=4),
                )

            for ch in range(NCHUNK):
                emit_chunk(ch)
                if ch == 0:
                    nc.gpsimd.memset(ad_bufs[1][:, : GC * 64], 0.0)
                    nc.gpsimd.memset(ad_bufs[1][:, GC * 64 :], 0.0)
                if NCHUNK // 2 - 1 <= ch < NCHUNK // 2 + 3:
                    uu = 2 * (ch - (NCHUNK // 2 - 1))
                    emit_spill(0, NCHUNK // 2 - 1, [uu, uu + 1])
            chunk_ctx.close()
            cnpool = ctx.enter_context(tc.tile_pool(name="cn", bufs=2))
            ospool = ctx.enter_context(tc.tile_pool(name="osb", bufs=2))
            emit_spill(NCHUNK // 2, NCHUNK - 1, [0])
            emit_gather(0)
            emit_spill(NCHUNK // 2, NCHUNK - 1, [1])
            emit_gather(1)
            emit_spill(NCHUNK // 2, NCHUNK - 1, range(2, 8))
            for j in range(8):
                if j + 2 < 8:
                    emit_gather(j + 2)
                emit_wave(j)

    _split_sync_waits(nc, limit=1)
    return nc


_NC_CACHE = {}


def _get_nc(T, SPAN):
    key = (T, SPAN)
    if key not in _NC_CACHE:
        _NC_CACHE[key] = build_kernel(T, SPAN)
    return _NC_CACHE[key]


def _prep_weights(w_qkv, b_qkv, w_out, b_out):
    bf = ml_dtypes.bfloat16
    w3 = w_qkv.reshape(H, 192, C).astype(np.float32)
    qw = (w3[:, :DK, :] / 8.0).reshape(H * DK, C)
    kw = w3[:, DK : 2 * DK, :].reshape(H * DK, C)
    vw = w3[:, 2 * DK :, :].reshape(H * DK, C)
    wqT = np.concatenate([qw, kw, vw], axis=0).T.copy().astype(bf)  # (C, 3072)
    b3 = b_qkv.reshape(H, 192).astype(np.float32)
    bq = np.concatenate(
        [(b3[:, :DK] / 8.0).reshape(-1), b3[:, DK : 2 * DK].reshape(-1), b3[:, 2 * DK :].reshape(-1)]
    )
    bq_d = bq.reshape(24, 128).T.copy().astype(np.float32)          # (128, 24)
    woT = w_out.T.copy().astype(bf)   # rows = context channels (h,d) h-major
    boT = b_out.reshape(1, C).astype(bf)
    return wqT, bq_d, woT, boT


def kernel(x, w_qkv, b_qkv, w_out, b_out, _trace=False, _span=256):
    B, _, T = x.shape
    assert B == N_CORES
    nc = _get_nc(T, _span)
    wqT, bq_d, woT, boT = _prep_weights(w_qkv, b_qkv, w_out, b_out)
    bf = ml_dtypes.bfloat16
    in_maps = []
    for b in range(B):
        in_maps.append(
            {
                "x": x[b].astype(bf),
                "wqT": wqT,
                "bqT": bq_d,
                "woT": woT,
                "boT": boT,
            }
        )
    res = run_bass_kernel_spmd(nc, in_maps, list(range(N_CORES)), trace=_trace)
    out = np.stack(
        [res.results[b]["outT"].astype(np.float32).T for b in range(B)], axis=0
    )
    if _trace:
        kernel.last_exec_time_ns = res.exec_time_ns
        kernel.last_results = res
    return out
